# revision 4
# baseline (speedup 1.0000x reference)
"""Butterworth IIR (order 4) over [B=128, T=160000, 1] on 8 TRN2 NeuronCores.

Strategy: a stable IIR's impulse response decays geometrically (max pole
radius ~0.668 here), so the filter is numerically exactly (tail < 3e-23)
a 128-tap causal FIR:  y[t] = sum_{k<128} h[k] x[t-k].

Chunking time into 128-sample chunks, with X[c, m] = x[c*128 + m]:
    y[c*128 + j] = sum_m X[c, m] H0[m, j] + sum_m X[c-1, m] H1[m, j]
    H0[m, j] = h[j - m]        (0 <= j - m < 128)
    H1[m, j] = h[j - m + 128]  (0 <= j - m + 128 < 128)

On device this is two accumulating TensorE matmuls per window with the
small fixed H matrices as the stationary operand and a phase-major
(transposed) view of x as the wide moving operand (N up to 512 chunks).

The kernel is HBM-bound: ~10.2MB of traffic vs a measured ~410GB/s
per-core aggregate DMA cap (any 2-3 queues together saturate it), so
the stream floor is ~25us.  On top of that the runtime adds ~17us of
fixed overhead (engine start/iram ~6.5us head; a 256-semaphore reset
sweep + barriers ~9us tail) that no kernel content can remove.

Schedule (v2): all three data queues (sync HWDGE, scalar HWDGE,
gpsimd SWDGE) stream continuously with ~3.2-3.5MB each, entries in
need order:
  * inputs striped so sequences land in compute order (seq0 split in
    two window-size pieces so the first matmul starts ASAP),
  * outputs for seqs 0-9 written as 2-seq groups from contiguous
    pair tiles (5000B rows -> better per-queue rate, fewer ~0.7us
    trigger instructions), seqs 10-15 as singles for fine-grained
    queue balancing at the tail,
  * gpsimd's queue is loaded lightest so its slow SWDGE drain
    overlaps the tail of the HWDGE streams.
f16 I/O halves traffic vs f32 (rounding adds ~3.7e-4 rel err; the
gate is 2e-2).  PSUM evacuation (f32->f16 cast) is split between the
scalar (ACT) and vector (DVE) engines; filler matmuls on the resident
H tile bridge the input-limited ramp so the PE clock gate reaches
2.4GHz early.

Sharding: pure data-parallel, batch 128 -> 16 sequences per core.
"""

import numpy as np

B_FULL = 128
T_FULL = 160000
N_CORES = 8
SEQ_PER_CORE = B_FULL // N_CORES  # 16
CHUNK = 128
NCHUNK = T_FULL // CHUNK  # 1250
TAPS = 128
NWIN = 512  # windows 512,512,226
SEQ_COLS = NCHUNK + 1  # 1251: col 0 is the zero predecessor chunk

_NC_CACHE = {}


def _impulse_response(b, a, n):
    """First n samples of the IIR impulse response, computed in float64
    via the same direct-form II transposed recurrence as the reference."""
    b = np.asarray(b, np.float64)
    a = np.asarray(a, np.float64)
    bn = b / a[0]
    an = a / a[0]
    order = len(a) - 1
    z = np.zeros(order, np.float64)
    h = np.zeros(n, np.float64)
    xt = 1.0
    for t in range(n):
        yt = bn[0] * xt + z[0]
        znew = np.empty_like(z)
        znew[:-1] = z[1:] + xt * bn[1:-1] - yt * an[1:-1]
        znew[-1] = xt * bn[-1] - yt * an[-1]
        z = znew
        h[t] = yt
        xt = 0.0
    return h


def _build_h_matrices(b, a):
    h = _impulse_response(b, a, TAPS)
    m = np.arange(CHUNK)[:, None]
    j = np.arange(CHUNK)[None, :]
    d0 = j - m
    d1 = j - m + CHUNK
    H0 = np.where((d0 >= 0) & (d0 < TAPS), h[np.clip(d0, 0, TAPS - 1)], 0.0)
    H1 = np.where((d1 >= 0) & (d1 < TAPS), h[np.clip(d1, 0, TAPS - 1)], 0.0)
    return np.concatenate([H0, H1], axis=1).astype(np.float16)  # [128, 256]


# Output grouping: pairs for seqs 0-9 (contiguous 2-seq tiles), singles
# for 10-15. Must match between _build_nc and _assemble_output.
OUT_GROUPS = [(0, 1), (2, 3), (4, 5), (6, 7), (8, 9),
              (10,), (11,), (12,), (13,), (14,), (15,)]


def _build_nc():
    import concourse.bacc as bacc
    import concourse.mybir as mybir
    from concourse.tile import TileContext

    f32 = mybir.dt.float32
    f16 = mybir.dt.float16
    nc = bacc.Bacc()
    xt = nc.declare_dram_parameter(
        "xt", [CHUNK, SEQ_PER_CORE * SEQ_COLS], f16, isOutput=False
    )
    hh = nc.declare_dram_parameter("hh", [CHUNK, 2 * CHUNK], f16, isOutput=False)
    yt = nc.declare_dram_parameter(
        "yt", [CHUNK, SEQ_PER_CORE * NCHUNK], f16, isOutput=True
    )

    wins = list(range(0, NCHUNK, NWIN))  # [0, 512, 1024]

    with TileContext(nc) as tc:
        with (
            tc.tile_pool(name="const", bufs=1) as cpool,
            tc.tile_pool(name="yout", bufs=1) as ypool,
            tc.tile_pool(name="acc", bufs=8, space="PSUM") as pspool,
        ):
            h_tile = cpool.tile([CHUNK, 2 * CHUNK], f16, name="h_tile")
            x_tile = cpool.tile(
                [CHUNK, SEQ_PER_CORE * SEQ_COLS], f16, name="x_tile"
            )
            S, A, G = nc.sync, nc.scalar, nc.gpsimd

            def in_dma(eng, s_lo, s_hi, c_lo=0, c_hi=None):
                # load columns [c_lo, c_hi) of seqs [s_lo, s_hi) (contiguous)
                lo = s_lo * SEQ_COLS + c_lo
                hi = s_hi * SEQ_COLS if c_hi is None else s_lo * SEQ_COLS + c_hi
                eng.dma_start(out=x_tile[:, lo:hi], in_=xt[:, lo:hi])

            # Input schedule, striped so sequences land in compute order.
            # seq0 arrives in two pieces so the first matmul starts ASAP.
            cut = NWIN + 1
            in_dma(S, 0, 1, 0, cut)        # seq0 first window (+zero col)
            A.dma_start(out=h_tile[:], in_=hh[:])
            in_dma(G, 2, 3)
            in_dma(S, 0, 1, cut, SEQ_COLS)  # rest of seq0
            in_dma(A, 1, 2)
            in_dma(S, 3, 5)
            in_dma(A, 5, 7)
            in_dma(G, 7, 9)
            in_dma(S, 9, 11)
            in_dma(A, 11, 13)
            in_dma(G, 13, 15)
            in_dma(G, 15, 16)

            # Per-group contiguous output tiles (evac writes slices; one
            # DMA per group once every member sequence is evacuated).
            y_tiles = {}
            for g in OUT_GROUPS:
                y_tiles[g] = ypool.tile(
                    [CHUNK, len(g) * NCHUNK], f16, name=f"y{g[0]}"
                )
            seq_group = {}
            for g in OUT_GROUPS:
                for k, s in enumerate(g):
                    seq_group[s] = (g, k)

            # Output queue + order (by readiness, balanced bytes, gpsimd
            # loaded lightest so its SWDGE drain overlaps the stream tail).
            out_sched = {
                0: [(0, 1), (8, 9), (12,), (15,)],          # sync
                1: [(2, 3), (6, 7), (13,)],                 # scalar
                2: [(4, 5), (10,), (11,), (14,)],           # gpsimd
            }
            out_eng = {}
            for qi, groups in out_sched.items():
                for g in groups:
                    out_eng[g] = (S, A, G)[qi]
            assert sorted(out_eng) == sorted(OUT_GROUPS)

            # Filler matmuls on the already-resident H tile keep the PE
            # array busy through the input-limited ramp: the HAM clock
            # gate needs ~3.4us of sustained activity to unlock 2.4GHz,
            # and any multi-us idle gap resets it. Fillers write garbage
            # into the NEXT real PSUM tile; the following start=True
            # matmul overwrites it, so no PSUM bank is wasted on scratch.
            def filler(p, n_mm, cols=CHUNK):
                for _ in range(n_mm):
                    nc.tensor.matmul(
                        p[:, :cols],
                        h_tile[:, 0:CHUNK],
                        h_tile[:, :cols],
                        start=True,
                        stop=True,
                    )

            for s in range(SEQ_PER_CORE):
                base = s * SEQ_COLS
                g, k = seq_group[s]
                y_tile = y_tiles[g]
                yoff = k * NCHUNK
                for wi, w in enumerate(wins):
                    n = min(NWIN, NCHUNK - w)
                    p = pspool.tile([CHUNK, NWIN], f32, name="p")
                    if s == 0 and wi == 0:
                        filler(p, 2, 2 * CHUNK)
                    elif s < 3:
                        filler(p, 3)
                    nc.tensor.matmul(
                        p[:, :n],
                        h_tile[:, 0:CHUNK],
                        x_tile[:, base + w + 1 : base + w + 1 + n],
                        start=True,
                        stop=False,
                    )
                    nc.tensor.matmul(
                        p[:, :n],
                        h_tile[:, CHUNK : 2 * CHUNK],
                        x_tile[:, base + w : base + w + n],
                        start=False,
                        stop=True,
                    )
                    # evacuate immediately (f32 -> f16 cast): ACT takes w0,
                    # DVE w1, and they split the short last window.
                    if wi == 0:
                        nc.scalar.copy(out=y_tile[:, yoff + w : yoff + w + n], in_=p[:, :n])
                    elif wi == 1:
                        nc.vector.tensor_copy(out=y_tile[:, yoff + w : yoff + w + n], in_=p[:, :n])
                    else:
                        h2 = n // 2
                        nc.scalar.copy(out=y_tile[:, yoff + w : yoff + w + h2], in_=p[:, :h2])
                        nc.vector.tensor_copy(
                            out=y_tile[:, yoff + w + h2 : yoff + w + n], in_=p[:, h2:n]
                        )
                # once the whole group is evacuated, stream it out
                if s == g[-1]:
                    out_eng[g].dma_start(
                        out=yt[:, g[0] * NCHUNK : (g[-1] + 1) * NCHUNK],
                        in_=y_tile[:],
                    )
    nc.compile()
    return nc


def _run_on_device(in_maps, trace=False):
    from concourse.bass_utils import run_bass_kernel_spmd

    if "nc" not in _NC_CACHE:
        _NC_CACHE["nc"] = _build_nc()
    return run_bass_kernel_spmd(
        _NC_CACHE["nc"], in_maps, core_ids=list(range(N_CORES)), trace=trace
    )


def _prepare_in_maps(x, b, a):
    hh = _build_h_matrices(b, a)
    xs = np.ascontiguousarray(np.asarray(x, np.float32).reshape(B_FULL, T_FULL))
    in_maps = []
    for c in range(N_CORES):
        xc = xs[c * SEQ_PER_CORE : (c + 1) * SEQ_PER_CORE]
        # phase-major: xt[p, s*1251 + 1 + c'] = x[s, c'*128 + p]; col 0 of
        # each sequence block is zeros (the "previous chunk" of chunk 0).
        xt = np.zeros((CHUNK, SEQ_PER_CORE, SEQ_COLS), np.float16)
        xt[:, :, 1:] = xc.reshape(SEQ_PER_CORE, NCHUNK, CHUNK).transpose(2, 0, 1)
        in_maps.append({"xt": np.ascontiguousarray(xt.reshape(CHUNK, -1)), "hh": hh})
    return in_maps


def _assemble_output(results):
    out = np.empty((B_FULL, T_FULL, 1), np.float32)
    for c in range(N_CORES):
        ytc = np.asarray(results[c]["yt"]).reshape(CHUNK, SEQ_PER_CORE, NCHUNK)
        yc = ytc.transpose(1, 2, 0).reshape(SEQ_PER_CORE, T_FULL)
        out[c * SEQ_PER_CORE : (c + 1) * SEQ_PER_CORE, :, 0] = yc.astype(np.float32)
    return out


def kernel(x, b, a):
    in_maps = _prepare_in_maps(x, b, a)
    res = _run_on_device(in_maps, trace=False)
    return _assemble_output(res.results)


def kernel_traced(x, b, a):
    """Same as kernel() but with neuron profiling; returns (output, exec_time_ns)."""
    in_maps = _prepare_in_maps(x, b, a)
    try:
        res = _run_on_device(in_maps, trace=True)
    except ModuleNotFoundError:
        res = _run_on_device(in_maps, trace=False)
    return _assemble_output(res.results), res.exec_time_ns


# revision 5
# speedup vs baseline: 1.1801x; 1.1801x over previous
"""Butterworth IIR (order 4) over [B=128, T=160000, 1] on 8 TRN2 NeuronCores.

Strategy: a stable IIR's impulse response decays geometrically (max pole
radius ~0.668 here), so the filter is numerically exactly (tail < 3e-23)
a 128-tap causal FIR:  y[t] = sum_{k<128} h[k] x[t-k].

Chunking time into 128-sample chunks, with X[c, m] = x[c*128 + m]:
    y[c*128 + j] = sum_m X[c, m] H0[m, j] + sum_m X[c-1, m] H1[m, j]
    H0[m, j] = h[j - m]        (0 <= j - m < 128)
    H1[m, j] = h[j - m + 128]  (0 <= j - m + 128 < 128)

On device this is two accumulating TensorE matmuls per window with the
small fixed H matrices as the stationary operand and a phase-major
(transposed) view of x as the wide moving operand (N up to 512 chunks).

Measured machine model (from ntff profiles of this kernel):
  * per-core aggregate DMA cap ~410-420GB/s; 2 HWDGE queues reach it
    on the input (read) phase, the output phase needs all three data
    queues (sync/scalar HWDGE + gpsimd SWDGE) to get close.
  * ~17us of fixed runtime overhead per launch (engine start + iram
    ~6.5us head; 256-semaphore reset sweep + barriers ~9us tail).
  * PE at full clock does ~1.06us/sequence; the HAM clock gate needs
    a few us of sustained PE activity before it unlocks 2.4GHz.

Schedule (v3):
  * f16 I/O: 10.2MB/core total traffic (gate is 2e-2 rel err, f16
    rounding costs 3.7e-4), stream floor ~25us.
  * inputs on the two HWDGE queues only (singles first so compute
    starts early, pairs later; seq0 in three window-size pieces).
  * warmup filler matmuls on the resident H tile run before the first
    input lands (hidden behind the DMA ramp) so the PE clock is at
    2.4GHz when real work starts; no fillers inside the real stream
    (interleaved fillers cost ~6us of PE time in the previous rev).
  * outputs as single-sequence tiles: evens on gpsimd (SWDGE), odds
    on sync, the last three sequences on scalar with their triggers
    emitted after all evacuation ops (a DMA trigger stuck waiting on
    an evac engine convoys everything behind it).
  * PSUM evacuation (f32 -> f16 cast) split between scalar (ACT) and
    vector (DVE).

Sharding: pure data-parallel, batch 128 -> 16 sequences per core.
"""

import numpy as np

B_FULL = 128
T_FULL = 160000
N_CORES = 8
SEQ_PER_CORE = B_FULL // N_CORES  # 16
CHUNK = 128
NCHUNK = T_FULL // CHUNK  # 1250
TAPS = 128
NWIN = 512  # windows 512,512,226
SEQ_COLS = NCHUNK + 1  # 1251: col 0 is the zero predecessor chunk

_NC_CACHE = {}


def _impulse_response(b, a, n):
    """First n samples of the IIR impulse response, computed in float64
    via the same direct-form II transposed recurrence as the reference."""
    b = np.asarray(b, np.float64)
    a = np.asarray(a, np.float64)
    bn = b / a[0]
    an = a / a[0]
    order = len(a) - 1
    z = np.zeros(order, np.float64)
    h = np.zeros(n, np.float64)
    xt = 1.0
    for t in range(n):
        yt = bn[0] * xt + z[0]
        znew = np.empty_like(z)
        znew[:-1] = z[1:] + xt * bn[1:-1] - yt * an[1:-1]
        znew[-1] = xt * bn[-1] - yt * an[-1]
        z = znew
        h[t] = yt
        xt = 0.0
    return h


def _build_h_matrices(b, a):
    h = _impulse_response(b, a, TAPS)
    m = np.arange(CHUNK)[:, None]
    j = np.arange(CHUNK)[None, :]
    d0 = j - m
    d1 = j - m + CHUNK
    H0 = np.where((d0 >= 0) & (d0 < TAPS), h[np.clip(d0, 0, TAPS - 1)], 0.0)
    H1 = np.where((d1 >= 0) & (d1 < TAPS), h[np.clip(d1, 0, TAPS - 1)], 0.0)
    return np.concatenate([H0, H1], axis=1).astype(np.float16)  # [128, 256]


def _build_nc():
    import concourse.bacc as bacc
    import concourse.mybir as mybir
    from concourse.tile import TileContext

    f32 = mybir.dt.float32
    f16 = mybir.dt.float16
    nc = bacc.Bacc()
    xt = nc.declare_dram_parameter(
        "xt", [CHUNK, SEQ_PER_CORE * SEQ_COLS], f16, isOutput=False
    )
    hh = nc.declare_dram_parameter("hh", [CHUNK, 2 * CHUNK], f16, isOutput=False)
    yt = nc.declare_dram_parameter(
        "yt", [CHUNK, SEQ_PER_CORE * NCHUNK], f16, isOutput=True
    )

    wins = list(range(0, NCHUNK, NWIN))  # [0, 512, 1024]

    with TileContext(nc) as tc:
        with (
            tc.tile_pool(name="const", bufs=1) as cpool,
            tc.tile_pool(name="yout", bufs=SEQ_PER_CORE) as ypool,
            tc.tile_pool(name="acc", bufs=8, space="PSUM") as pspool,
        ):
            h_tile = cpool.tile([CHUNK, 2 * CHUNK], f16, name="h_tile")
            x_tile = cpool.tile(
                [CHUNK, SEQ_PER_CORE * SEQ_COLS], f16, name="x_tile"
            )
            S, A = nc.sync, nc.scalar

            def in_dma(eng, lo, hi):
                eng.dma_start(out=x_tile[:, lo:hi], in_=xt[:, lo:hi])

            # Input schedule on the two HWDGE queues (they saturate the
            # ~410GB/s core DMA cap on reads). h first on scalar so the
            # PE warmup fillers can start ~7us; seq0 in window pieces so
            # the first real matmul starts as soon as possible.
            A.dma_start(out=h_tile[:], in_=hh[:])
            cuts = [0] + [w + min(NWIN, NCHUNK - w) + 1 for w in wins]
            piece_engs = [S, A, S]
            for eng, (lo, hi) in zip(piece_engs, zip(cuts[:-1], cuts[1:])):
                in_dma(eng, lo, hi)
            # singles for seqs 1-5 (keeps early compute fed at fine
            # granularity), pairs for 6-15 (fewer trigger instructions).
            in_sched = [
                (A, 1, 2), (S, 2, 3), (A, 3, 4), (S, 4, 5), (A, 5, 6),
                (S, 6, 8), (A, 8, 10), (S, 10, 12), (A, 12, 14), (S, 14, 16),
            ]
            for eng, s_lo, s_hi in in_sched:
                in_dma(eng, s_lo * SEQ_COLS, s_hi * SEQ_COLS)

            # Output queue per sequence: evens -> gpsimd (SWDGE), odds ->
            # sync, except the last three go to scalar AFTER all its evac
            # work (triggers emitted post-loop so they can never convoy
            # the evacuation stream).
            SCALAR_TAIL = (13, 14, 15)

            # Warmup fillers: ~2.4us of matmuls on the resident H tile,
            # runnable as soon as h lands (~7us) and finished by the time
            # seq0's first window arrives (~9.5us). They earn the HAM
            # 2.4GHz clock unlock before real work starts and cost no
            # critical-path time. They write a scratch PSUM tile.
            warm = pspool.tile([CHUNK, NWIN], f32, name="p")
            for _ in range(14):
                nc.tensor.matmul(
                    warm[:, : 2 * CHUNK],
                    h_tile[:, 0:CHUNK],
                    h_tile[:],
                    start=True,
                    stop=True,
                )

            y_tiles = []
            for s in range(SEQ_PER_CORE):
                base = s * SEQ_COLS
                y_tile = ypool.tile([CHUNK, NCHUNK], f16, name="y_tile")
                y_tiles.append(y_tile)
                for wi, w in enumerate(wins):
                    n = min(NWIN, NCHUNK - w)
                    p = pspool.tile([CHUNK, NWIN], f32, name="p")
                    nc.tensor.matmul(
                        p[:, :n],
                        h_tile[:, 0:CHUNK],
                        x_tile[:, base + w + 1 : base + w + 1 + n],
                        start=True,
                        stop=False,
                    )
                    nc.tensor.matmul(
                        p[:, :n],
                        h_tile[:, CHUNK : 2 * CHUNK],
                        x_tile[:, base + w : base + w + n],
                        start=False,
                        stop=True,
                    )
                    # evacuate immediately (f32 -> f16 cast): ACT takes w0,
                    # DVE w1, and they split the short last window.
                    if wi == 0:
                        nc.scalar.copy(out=y_tile[:, w : w + n], in_=p[:, :n])
                    elif wi == 1:
                        nc.vector.tensor_copy(out=y_tile[:, w : w + n], in_=p[:, :n])
                    else:
                        h2 = n // 2
                        nc.scalar.copy(out=y_tile[:, w : w + h2], in_=p[:, :h2])
                        nc.vector.tensor_copy(
                            out=y_tile[:, w + h2 : w + n], in_=p[:, h2:n]
                        )
                if s not in SCALAR_TAIL:
                    out_eng = nc.gpsimd if s % 2 == 0 else nc.sync
                    out_eng.dma_start(
                        out=yt[:, s * NCHUNK : (s + 1) * NCHUNK], in_=y_tile[:]
                    )
            for s in SCALAR_TAIL:
                nc.scalar.dma_start(
                    out=yt[:, s * NCHUNK : (s + 1) * NCHUNK], in_=y_tiles[s][:]
                )
    nc.compile()
    return nc


def _run_on_device(in_maps, trace=False):
    from concourse.bass_utils import run_bass_kernel_spmd

    if "nc" not in _NC_CACHE:
        _NC_CACHE["nc"] = _build_nc()
    return run_bass_kernel_spmd(
        _NC_CACHE["nc"], in_maps, core_ids=list(range(N_CORES)), trace=trace
    )


def _prepare_in_maps(x, b, a):
    hh = _build_h_matrices(b, a)
    xs = np.ascontiguousarray(np.asarray(x, np.float32).reshape(B_FULL, T_FULL))
    in_maps = []
    for c in range(N_CORES):
        xc = xs[c * SEQ_PER_CORE : (c + 1) * SEQ_PER_CORE]
        # phase-major: xt[p, s*1251 + 1 + c'] = x[s, c'*128 + p]; col 0 of
        # each sequence block is zeros (the "previous chunk" of chunk 0).
        xt = np.zeros((CHUNK, SEQ_PER_CORE, SEQ_COLS), np.float16)
        xt[:, :, 1:] = xc.reshape(SEQ_PER_CORE, NCHUNK, CHUNK).transpose(2, 0, 1)
        in_maps.append({"xt": np.ascontiguousarray(xt.reshape(CHUNK, -1)), "hh": hh})
    return in_maps


def _assemble_output(results):
    out = np.empty((B_FULL, T_FULL, 1), np.float32)
    for c in range(N_CORES):
        ytc = np.asarray(results[c]["yt"]).reshape(CHUNK, SEQ_PER_CORE, NCHUNK)
        yc = ytc.transpose(1, 2, 0).reshape(SEQ_PER_CORE, T_FULL)
        out[c * SEQ_PER_CORE : (c + 1) * SEQ_PER_CORE, :, 0] = yc.astype(np.float32)
    return out


def kernel(x, b, a):
    in_maps = _prepare_in_maps(x, b, a)
    res = _run_on_device(in_maps, trace=False)
    return _assemble_output(res.results)


def kernel_traced(x, b, a):
    """Same as kernel() but with neuron profiling; returns (output, exec_time_ns)."""
    in_maps = _prepare_in_maps(x, b, a)
    try:
        res = _run_on_device(in_maps, trace=True)
    except ModuleNotFoundError:
        res = _run_on_device(in_maps, trace=False)
    return _assemble_output(res.results), res.exec_time_ns
